# revision 9
# baseline (speedup 1.0000x reference)
"""TRN2 Bass kernel v3 for 16-head MHA (B=2, T=2048, D=1024).

Sharding: batch x head-quad across 8 cores (core = 4*b + hq handles batch b,
heads 4*hq..4*hq+3).  Per core: bf16 Q/K projections, bf16 V projection in
[token, dim] layout (no transposes; bv folded into the host-side output
bias), quadrant-packed bf16 S^T matmuls, softmax exp split across
ACT/DVE/GPSIMD (DVE+GP use an exp bit-trick producing bf16 bits), PV via
fp8 DoubleRow for head-pair 0 (PV8 flag per group) and bf16 for the rest,
normalization via a PE ones-x-recip broadcast matmul, bf16 output
projection, bf16 rank-256 partial written to DRAM.  Host sums 4 partials
per batch and adds bo + bv@Wo.
"""

import math
import numpy as np

import concourse.bass as bass
import concourse.mybir as mybir
import concourse.tile as tile
from concourse import bacc

FP32 = mybir.dt.float32
BF16 = mybir.dt.bfloat16
F8 = mybir.dt.float8e4
I8 = mybir.dt.int8
I16 = mybir.dt.int16
DR = mybir.MatmulPerfMode.DoubleRow
ACT_EXP = mybir.ActivationFunctionType.Exp
ACT_ID = mybir.ActivationFunctionType.Identity
OP = mybir.AluOpType

T = 2048          # tokens per core (one batch)
NG = T // 512     # q groups
LN2 = math.log(2)
# fp8 DoubleRow PV for head-pair 0, per group (flip entries to trade err/speed)
PV8 = [True, True, False, False]
# DVE trick-exp constants (truncating float->int convert: +0.5 for rounding);
# S in [-7, 7] keeps t inside [0, 32639] so no clip is needed
A16, B16 = 128.0 / LN2, 16256.0 + 0.5 - 5.5
# engine pattern for head-pair-1 exp units (hp0 fp8 units always go to ACT);
# 't' = single-op DVE bit-trick (mul-add, int16 convert = bf16 bits)
HP1_PAT = ['a', 't'] * 8


def build(nc=None):
    if nc is None:
        nc = bacc.Bacc(
            "TRN2",
            target_bir_lowering=False,
            debug=False,
            enable_asserts=False,
            num_devices=8,
        )

    xqb = nc.dram_tensor("xqb", [128, 8, T], BF16, kind="ExternalInput")
    xkb = nc.dram_tensor("xkb", [128, 8, T], BF16, kind="ExternalInput")
    xvb = nc.dram_tensor("xvb", [128, 8, T], BF16, kind="ExternalInput")
    wqb = nc.dram_tensor("wqb", [128, 8, 256], BF16, kind="ExternalInput")
    wkb = nc.dram_tensor("wkb", [128, 8, 256], BF16, kind="ExternalInput")
    wvb = nc.dram_tensor("wvb", [128, 8, 256], BF16, kind="ExternalInput")
    wob = nc.dram_tensor("wob", [128, 2, 1024], BF16, kind="ExternalInput")
    bq8 = nc.dram_tensor("bq8", [128, 2], FP32, kind="ExternalInput")
    bkt = nc.dram_tensor("bkt", [128, 2], FP32, kind="ExternalInput")
    out = nc.dram_tensor("out", [T, 1024], BF16, kind="ExternalOutput")

    with tile.TileContext(nc) as tc:
        _emit(nc, tc, xqb, xkb, xvb, wqb, wkb, wvb, wob, bq8, bkt, out)

    nc.compile()
    return nc


class _E:
    pass


def _emit(nc, tc, xqb, xkb, xvb, wqb, wkb, wvb, wob, bq8, bkt, out):
    from contextlib import ExitStack

    E = _E()
    E.nc = nc
    E.ucount = 0   # hp1 exp unit counter
    E.opc = 0      # outproj chunks already emitted for pend[0]

    with ExitStack() as ctx:
        const = ctx.enter_context(tc.tile_pool(name="const", bufs=1))
        big = ctx.enter_context(tc.tile_pool(name="big", bufs=1))
        E.pt8_pool = ctx.enter_context(tc.tile_pool(name="pt8", bufs=3))
        E.ptb_pool = ctx.enter_context(tc.tile_pool(name="ptb", bufs=4))
        E.rc_pool = ctx.enter_context(tc.tile_pool(name="rc", bufs=2))
        E.ostg_pool = ctx.enter_context(tc.tile_pool(name="ostg", bufs=3))
        # PSUM: po 2 + st 2x2 + ctx 2x1 = 8 banks
        E.po_ps = ctx.enter_context(tc.tile_pool(name="po_ps", bufs=2, space="PSUM"))
        E.st_ps = ctx.enter_context(tc.tile_pool(name="st_ps", bufs=2, space="PSUM"))
        E.ctx_ps = ctx.enter_context(tc.tile_pool(name="ctx_ps", bufs=2, space="PSUM"))

        # ---- constants / weights ----
        wk_sb = const.tile([128, 8, 256], BF16, tag="wk")
        wq_sb = const.tile([128, 8, 256], BF16, tag="wq")
        wv_sb = const.tile([128, 8, 256], BF16, tag="wv")
        E.wo_sb = const.tile([128, 2, 1024], BF16, tag="wo")
        bq_sb = const.tile([128, 2], FP32, tag="bq")
        bk_sb = const.tile([128, 2], FP32, tag="bk")
        E.ebias = const.tile([128, 1], FP32, tag="ebias")
        E.ones = const.tile([128, 64], BF16, tag="ones")
        nc.sync.dma_start(wk_sb[:], wkb.ap())
        nc.gpsimd.memset(E.ebias[:], float(-2 * LN2))
        nc.gpsimd.memset(E.ones[:], 1.0)

        # ---- persistent activations ----
        xk_sb = big.tile([128, 8, T], BF16, tag="xk")
        xq_sb = big.tile([128, 8, T], BF16, tag="xq")
        xv_sb = big.tile([128, 8, T], BF16, tag="xv")
        E.qT = big.tile([128, 2, T], BF16, tag="qT")
        E.kT = big.tile([128, 2, T], BF16, tag="kT")
        E.ctxT = big.tile([128, 2, T], BF16, tag="ctxT")
        # v8: [p, c2, hl, i, 65] fp8 (heads 0,1); vb: [p, tk, h, 65] bf16 (all 4)
        E.v8 = big.tile([128, 8, 2, 2, 80], F8, tag="v8")  # 80: 16B-aligned DoubleRow pair stride
        E.vb = big.tile([128, 16, 4, 65], BF16, tag="vb")

        # chunked input DMAs so projections start before full tensors land
        for kc in range(8):
            nc.sync.dma_start(xk_sb[:, kc, :], xkb.ap()[:, kc, :])
        nc.sync.dma_start(bk_sb[:], bkt.ap())
        nc.sync.dma_start(wq_sb[:], wqb.ap())
        for kc in range(8):
            nc.sync.dma_start(xq_sb[:, kc, :], xqb.ap()[:, kc, :])
        nc.sync.dma_start(bq_sb[:], bq8.ap())
        nc.sync.dma_start(wv_sb[:], wvb.ap())
        for kc in range(8):
            nc.sync.dma_start(xv_sb[:, kc, :], xvb.ap()[:, kc, :])
        nc.sync.dma_start(E.wo_sb[:], wob.ap())
        nc.gpsimd.memset(E.v8[:, :, :, :, 64], 1.0)
        nc.gpsimd.memset(E.vb[:, :, :, 64], 1.0)

        # ---- Q/K projections (bf16) ----
        def qkproj(x_sb, w_sb, dstT, bias_sb, scale):
            for s in range(2):
                for t in range(4):
                    ps = E.po_ps.tile([128, 512], FP32, tag="po", name="qk_ps")
                    for kc in range(8):
                        nc.tensor.matmul(
                            ps[:],
                            w_sb[:, kc, s * 128:(s + 1) * 128],
                            x_sb[:, kc, t * 512:(t + 1) * 512],
                            start=(kc == 0), stop=(kc == 7),
                        )
                    nc.scalar.activation(
                        dstT[:, s, t * 512:(t + 1) * 512], ps[:], ACT_ID,
                        bias=bias_sb[:, s:s + 1], scale=scale)

        qkproj(xk_sb, wk_sb, E.kT, bk_sb, 1.0)
        qkproj(xq_sb, wq_sb, E.qT, bq_sb, 0.125)

        # ---- V projection (bf16, direct [tok, dim] layout) ----
        for tc_i in range(16):
            vp = E.po_ps.tile([128, 256], FP32, tag="po", name="v_ps")
            for kc in range(8):
                nc.tensor.matmul(
                    vp[:],
                    xv_sb[:, kc, tc_i * 128:(tc_i + 1) * 128],
                    wv_sb[:, kc, :],
                    start=(kc == 0), stop=(kc == 7),
                )
            c2, i = tc_i // 2, tc_i % 2
            nc.vector.tensor_copy(E.vb[:, tc_i, :, 0:64], vp[:])
            nc.vector.tensor_copy(E.v8[:, c2, :, i, 0:64], vp[:, 0:128])

        # ---- attention groups ----
        pend = []
        for g in range(NG):
            _group(E, g, pend, out)
        while pend:
            _outproj(E, pend.pop(0), out, range(8))


def _exp_unit(E, st, fp8, pt8, ptb, i):
    """Exp of one st tile [128, 2, 512] into pt slot i."""
    nc = E.nc
    if fp8:
        nc.scalar.activation(pt8[:, :, i, :], st[:], ACT_EXP,
                             bias=E.ebias[:], scale=1.0)
        return
    dst = ptb[:, :, i, :]
    eng = HP1_PAT[E.ucount % len(HP1_PAT)]
    E.ucount += 1
    if eng == 'a':
        nc.scalar.activation(dst, st[:], ACT_EXP, scale=1.0)
    else:
        nc.vector.tensor_scalar(dst.bitcast(I16), st[:], float(A16), float(B16),
                                op0=OP.mult, op1=OP.add)


def _group(E, g, pend, out):
    nc = E.nc
    q0 = g * 512
    for hp in range(2):
        fp8 = (hp == 0) and PV8[g]
        ctx2 = [
            E.ctx_ps.tile([65, 512], FP32, tag="ctx", name=f"ctx{hl}")
            for hl in range(2)
        ]
        def pv(c2, pt):
            """PV matmuls for double-chunk c2 (software-pipelined one c2 behind)."""
            if fp8:
                for hl in range(2):
                    nc.tensor.matmul(
                        ctx2[hl][:],
                        E.v8[:, c2, hl, :, 0:65],
                        pt[:, hl, :, :],
                        start=(c2 == 0), stop=(c2 == 7), perf_mode=DR,
                    )
            else:
                for i in range(2):
                    tk = c2 * 2 + i
                    for hl in range(2):
                        nc.tensor.matmul(
                            ctx2[hl][:],
                            E.vb[:, tk, hp * 2 + hl, :],
                            pt[:, hl, i, :],
                            start=(tk == 0), stop=(tk == 15),
                        )

        prev = None
        for c2 in range(8):
            if fp8:
                pt = E.pt8_pool.tile([128, 2, 2, 512], F8, tag="pt8", name="pt8")
            else:
                pt = E.ptb_pool.tile([128, 2, 2, 512], BF16, tag="ptb", name="ptb")
            for i in range(2):
                tk = c2 * 2 + i
                st = E.st_ps.tile([128, 2, 512], FP32, tag="st", name="st")
                for hl in range(2):
                    nc.tensor.matmul(
                        st[:, hl, :],
                        E.kT[hl * 64:(hl + 1) * 64, hp, tk * 128:(tk + 1) * 128],
                        E.qT[hl * 64:(hl + 1) * 64, hp, q0:q0 + 512],
                        start=True, stop=True,
                    )
                _exp_unit(E, st, fp8, pt if fp8 else None, None if fp8 else pt, i)
            if prev is not None:
                pv(*prev)
            prev = (c2, pt)
            # keep PE fed: one deferred outproj chunk every other c2
            if pend and (c2 % 4 == 1):
                _outproj(E, pend[0], out, [E.opc])
                E.opc += 1
        pv(*prev)

        # ---- normalize + drain ctx for this head pair ----
        recipb = E.po_ps.tile([128, 512], FP32, tag="po", name="recipb")
        rcps = []
        for hl in range(2):
            rsrc = E.rc_pool.tile([1, 512], FP32, tag="rs", name=f"rs{hl}")
            rcpf = E.rc_pool.tile([1, 512], FP32, tag="rff", name=f"rff{hl}")
            rcp = E.rc_pool.tile([1, 512], BF16, tag="rf", name=f"rf{hl}")
            nc.vector.tensor_copy(rsrc[0:1, :], ctx2[hl][64:65, :])
            nc.vector.reciprocal_approx_fast(rcpf[0:1, :], rsrc[0:1, :])
            nc.vector.tensor_copy(rcp[0:1, :], rcpf[0:1, :])
            rcps.append(rcp)
        for hl in range(2):
            nc.tensor.matmul(
                recipb[hl * 64:(hl + 1) * 64, :],
                E.ones[0:1, :],
                rcps[hl][0:1, :],
                start=True, stop=True,
            )
        rb_sb = E.rc_pool.tile([128, 512], BF16, tag="rbsb", name="rb_sb")
        nc.vector.tensor_copy(rb_sb[:], recipb[:])
        for hl in range(2):
            nc.vector.tensor_tensor(
                E.ctxT[hl * 64:(hl + 1) * 64, hp, q0:q0 + 512],
                ctx2[hl][0:64, :],
                rb_sb[hl * 64:(hl + 1) * 64, :],
                op=OP.mult,
            )
    # finish any outproj chunks of the previous group not yet emitted
    if pend:
        gprev = pend.pop(0)
        _outproj(E, gprev, out, range(E.opc, 8))
    pend.append(g)
    E.opc = 0


def _outproj(E, g, out, chunks):
    nc = E.nc
    q0 = g * 512
    for ch in chunks:
        tc4, hh = ch // 2, ch % 2
        t0 = q0 + tc4 * 128
        ops = E.po_ps.tile([128, 512], FP32, tag="po", name="ops")
        for s in range(2):
            nc.tensor.matmul(
                ops[:],
                E.ctxT[:, s, t0:t0 + 128],
                E.wo_sb[:, s, hh * 512:(hh + 1) * 512],
                start=(s == 0), stop=(s == 1),
            )
        ostg = E.ostg_pool.tile([128, 512], BF16, tag="ostg")
        nc.scalar.activation(ostg[:], ops[:], mybir.ActivationFunctionType.Copy)
        nc.sync.dma_start(out.ap()[t0:t0 + 128, hh * 512:(hh + 1) * 512], ostg[:])


# ---------------- host-side helpers ----------------

def core_inputs(q, k, v, Wq, bq, Wk, bk, Wv, bv, Wo, core):
    import ml_dtypes
    bf = ml_dtypes.bfloat16
    f8 = ml_dtypes.float8_e4m3  # noqa: F841 (fp8 staging handled on device)
    b, hq = core // 4, core % 4
    sl = slice(hq * 256, (hq + 1) * 256)

    def kc8(x):   # [1024, N] -> [128, 8, N]
        return np.ascontiguousarray(
            x.reshape(8, 128, x.shape[1]).transpose(1, 0, 2))

    return {
        "xqb": kc8(np.asarray(q[b], np.float32).T).astype(bf),
        "xkb": kc8(np.asarray(k[b], np.float32).T).astype(bf),
        "xvb": kc8(np.asarray(v[b], np.float32).T).astype(bf),
        "wqb": kc8(np.ascontiguousarray(Wq[:, sl])).astype(bf),
        "wkb": kc8(np.ascontiguousarray(Wk[:, sl])).astype(bf),
        "wvb": kc8(np.ascontiguousarray(Wv[:, sl])).astype(bf),
        "wob": np.ascontiguousarray(
            Wo[sl, :].reshape(2, 128, 1024).transpose(1, 0, 2)).astype(bf),
        "bq8": np.ascontiguousarray(
            (bq[sl] / 8.0).reshape(2, 128).T).astype(np.float32),
        "bkt": np.ascontiguousarray(bk[sl].reshape(2, 128).T).astype(np.float32),
    }


def shared_inputs(q, k, v):
    return {}


# ---------------- public entry point ----------------

_NC_CACHE = []


def _get_nc():
    if not _NC_CACHE:
        _NC_CACHE.append(build())
    return _NC_CACHE[0]


def kernel(q, k, v, Wq, bq, Wk, bk, Wv, bv, Wo, bo):
    from concourse import bass_utils

    args = [np.asarray(a, np.float32) for a in (q, k, v, Wq, bq, Wk, bk, Wv, bv, Wo)]
    q, k, v, Wq, bq, Wk, bk, Wv, bv, Wo = args
    bo = np.asarray(bo, np.float32)

    nc = _get_nc()
    in_maps = [core_inputs(q, k, v, Wq, bq, Wk, bk, Wv, bv, Wo, core)
               for core in range(8)]
    res = bass_utils.run_bass_kernel_spmd(nc, in_maps, core_ids=list(range(8)))

    host_bias = bo.astype(np.float64) + bv.astype(np.float64) @ Wo.astype(np.float64)
    outp = np.zeros((2, T, 1024), np.float64)
    for core in range(8):
        outp[core // 4] += res.results[core]["out"].astype(np.float64)
    return (outp + host_bias).astype(np.float32)


# revision 10
# speedup vs baseline: 1.3100x; 1.3100x over previous
"""Self-contained TRN2 Bass kernel for 16-head MHA (B=2, T=2048, D=1024),
head-parallel across 8 NeuronCores (2 heads per core).

kernel(**inputs) takes the FULL fp32 inputs of reference.setup_inputs() and
returns the FULL [2, 2048, 1024] fp32 output.  Host-side prep: q/k/v are
transposed to [1024, 4096] bf16 (shared by all cores); each core gets its
128-column slice of Wq/Wk/Wv (and 128-row slice of Wo) in bf16.  Each core
computes its two heads end-to-end (QKV projections, softmax attention with
row-group-packed score matmuls, ones-augmented PV for free softmax sums,
output projection) and DMAs a rank-128 partial of the output back; the host
sums the 8 partials and adds the output bias.
"""

import numpy as np

import concourse.bass as bass
import concourse.mybir as mybir
import concourse.tile as tile
from concourse import bacc

FP32 = mybir.dt.float32
BF16 = mybir.dt.bfloat16

D = 1024          # model dim
N = 4096          # B*T tokens
B = 2
T = 2048
PH = 128          # per-core projection dims (2 heads x 64)
DH = 64           # head dim
KC = 8            # contraction chunks (1024 / 128)
NTC = N // 128    # 32 token chunks of 128
SCALE = 0.125     # 1/sqrt(64)

ACT_EXP = mybir.ActivationFunctionType.Exp


def build(nc=None):
    if nc is None:
        nc = bacc.Bacc(
            "TRN2",
            target_bir_lowering=False,
            debug=False,
            enable_asserts=False,
            num_devices=8,
        )

    qT = nc.dram_tensor("qT", [D, N], BF16, kind="ExternalInput")
    kT = nc.dram_tensor("kT", [D, N], BF16, kind="ExternalInput")
    vT = nc.dram_tensor("vT", [D, N], BF16, kind="ExternalInput")
    wq = nc.dram_tensor("wq", [D, PH], BF16, kind="ExternalInput")
    wk = nc.dram_tensor("wk", [D, PH], BF16, kind="ExternalInput")
    wv = nc.dram_tensor("wv", [D, PH], BF16, kind="ExternalInput")
    wo = nc.dram_tensor("wo", [PH, D], BF16, kind="ExternalInput")
    bq = nc.dram_tensor("bq", [PH, 1], FP32, kind="ExternalInput")
    bk = nc.dram_tensor("bk", [PH, 1], FP32, kind="ExternalInput")
    bv = nc.dram_tensor("bv", [PH, 1], FP32, kind="ExternalInput")
    out = nc.dram_tensor("out", [N, D], FP32, kind="ExternalOutput")

    with tile.TileContext(nc) as tc:
        _emit(nc, tc, qT, kT, vT, wq, wk, wv, wo, bq, bk, bv, out)

    nc.compile()
    return nc


class _Ctx:
    pass


def _emit(nc, tc, qT, kT, vT, wq, wk, wv, wo, bq, bk, bv, out):
    from contextlib import ExitStack

    E = _Ctx()
    E.nc = nc
    E.pending = []

    ctxmgr = ExitStack()
    with ctxmgr:
        const_pool = ctxmgr.enter_context(tc.tile_pool(name="const", bufs=1))
        E.xt_pool = ctxmgr.enter_context(tc.tile_pool(name="xt", bufs=18))
        big_pool = ctxmgr.enter_context(tc.tile_pool(name="big", bufs=1))
        E.pt_pool = ctxmgr.enter_context(tc.tile_pool(name="pt", bufs=17))
        E.bc_pool = ctxmgr.enter_context(tc.tile_pool(name="bc", bufs=4))
        E.ostg_pool = ctxmgr.enter_context(tc.tile_pool(name="ostg", bufs=4))
        # PSUM: shared proj/outproj pool 2 banks + st 4 + ctx 2 = 8
        E.po_ps = ctxmgr.enter_context(
            tc.tile_pool(name="po_ps", bufs=2, space="PSUM"))
        E.st_ps = ctxmgr.enter_context(
            tc.tile_pool(name="st_ps", bufs=2, space="PSUM"))
        E.ctx_ps = ctxmgr.enter_context(
            tc.tile_pool(name="ctx_ps", bufs=2, space="PSUM"))

        # --- weights / consts to SBUF ---
        wq_sb = const_pool.tile([128, KC, PH], BF16, tag="wq")
        wk_sb = const_pool.tile([128, KC, PH], BF16, tag="wk")
        wv_sb = const_pool.tile([128, KC, PH], BF16, tag="wv")
        E.wo_sb = const_pool.tile([128, D], BF16, tag="wo")
        bq_sb = const_pool.tile([128, 1], FP32, tag="bq")
        bk_sb = const_pool.tile([128, 1], FP32, tag="bk")
        E.bv_sb = const_pool.tile([128, 1], FP32, tag="bv")
        nc.sync.dma_start(wk_sb[:], wk.ap().rearrange("(c p) m -> p c m", p=128))
        nc.sync.dma_start(wq_sb[:], wq.ap().rearrange("(c p) m -> p c m", p=128))
        nc.sync.dma_start(wv_sb[:], wv.ap().rearrange("(c p) m -> p c m", p=128))
        nc.sync.dma_start(E.wo_sb[:], wo.ap())
        nc.sync.dma_start(bq_sb[:], bq.ap())
        nc.sync.dma_start(bk_sb[:], bk.ap())
        nc.sync.dma_start(E.bv_sb[:], bv.ap())

        # persistent activations
        E.qT_sb = big_pool.tile([128, N], BF16, tag="qTsb")
        E.kT_sb = big_pool.tile([128, N], BF16, tag="kTsb")
        # v_aug pair layout: [tok part, 32 tok chunks, 130]; per head h the
        # PV stationary operand is vp[:, chunk, 65h : 65h+65] = [v_h | ones]
        E.vp = big_pool.tile([128, NTC, 130], BF16, tag="vp")
        E.vT_sb = big_pool.tile([128, N], BF16, tag="vTsb")
        E.ctxT = big_pool.tile([128, N], BF16, tag="ctxT")

        nc.gpsimd.memset(E.vp[:, :, 64], 1.0)
        nc.gpsimd.memset(E.vp[:, :, 129], 1.0)

        E.identity = const_pool.tile([128, 128], BF16, tag="ident")
        from concourse.masks import make_identity
        make_identity(nc, E.identity[:])

        def dma_in(nm, xdram, b):
            lst = []
            for kc in range(KC):
                xt = E.xt_pool.tile(
                    [128, T], BF16, tag="xt", name=f"xt_{nm}{b}{kc}")
                nc.sync.dma_start(
                    xt[:], xdram.ap()[kc * 128:(kc + 1) * 128, b * T:(b + 1) * T])
                lst.append(xt)
            return lst

        def proj4(xts, wsb, bias_sb, dstT, b, ts=range(4), drain_act=False):
            for t in ts:
                _proj_chunk(E, xts, wsb, bias_sb, dstT, b * T, t, drain_act)

        def vproj(xts, b):
            proj4(xts, wv_sb, None, E.vT_sb, b)
            for tloc in range(16):
                tcid = b * 16 + tloc
                tr = E.po_ps.tile(
                    [128, 128], BF16, tag="po", name=f"tr{tcid}")
                nc.tensor.transpose(
                    tr[:], E.vT_sb[:, tcid * 128:(tcid + 1) * 128], E.identity[:])
                nc.vector.tensor_copy(E.vp[:, tcid, 0:64], tr[:, 0:64])
                nc.vector.tensor_copy(E.vp[:, tcid, 65:129], tr[:, 64:128])

        def group(b, tqc, mid_cb=None, defer=2):
            pend = _attention_group(E, b, tqc, mid_cb)
            while len(E.pending) >= defer:
                _norm_outproj(E, *E.pending.pop(0), out)
            E.pending.append(pend)

        # batch-0 inputs + K/Q projections up front; V is emitted after the
        # first attention group so the PE stream does not stall on vT DMA.
        # batch-1 inputs/projections are staggered into batch-0's ACT-bound
        # attention groups so their DMA + PE work hide in the slack.
        xk0 = dma_in("k", kT, 0)
        xq0 = dma_in("q", qT, 0)
        xv0 = dma_in("v", vT, 0)
        proj4(xk0, wk_sb, bk_sb, E.kT_sb, 0, drain_act=True)
        proj4(xq0, wq_sb, bq_sb, E.qT_sb, 0, drain_act=True)
        xk1 = dma_in("k", kT, 1)
        group(0, 0, mid_cb=lambda: vproj(xv0, 0))
        xq1 = dma_in("q", qT, 1)
        group(0, 1)
        proj4(xk1, wk_sb, bk_sb, E.kT_sb, 1, ts=(0, 1))
        xv1 = dma_in("v", vT, 1)
        group(0, 2)
        proj4(xk1, wk_sb, bk_sb, E.kT_sb, 1, ts=(2, 3))
        proj4(xq1, wq_sb, bq_sb, E.qT_sb, 1, ts=(0, 1))
        group(0, 3)
        proj4(xq1, wq_sb, bq_sb, E.qT_sb, 1, ts=(2, 3))
        group(1, 0, mid_cb=lambda: vproj(xv1, 1))
        group(1, 1)
        group(1, 2, defer=1)
        group(1, 3, defer=1)
        while E.pending:
            _norm_outproj(E, *E.pending.pop(0), out)


def _proj_chunk(E, xts, wsb, bias_sb, dstT, btok, t, drain_act=False):
    """One 512-token projection chunk: accumulate 8 kc matmuls, drain."""
    nc = E.nc
    ps = E.po_ps.tile([128, 512], FP32, tag="po", name="ps")
    for kc in range(KC):
        nc.tensor.matmul(
            ps[:],
            wsb[:, kc, :],
            xts[kc][:, t * 512:(t + 1) * 512],
            start=(kc == 0),
            stop=(kc == KC - 1),
        )
    dst = dstT[:, btok + t * 512: btok + (t + 1) * 512]
    if drain_act:
        # ScalarE drain (idle during the head phase); Identity has a free
        # per-partition bias add
        if bias_sb is not None:
            nc.scalar.activation(
                dst, ps[:], mybir.ActivationFunctionType.Identity, bias=bias_sb[:])
        else:
            nc.scalar.activation(dst, ps[:], mybir.ActivationFunctionType.Identity)
    elif bias_sb is not None:
        nc.vector.tensor_scalar_add(dst, ps[:], bias_sb[:])
    else:
        nc.vector.tensor_copy(dst, ps[:])


def _attention_group(E, b, tqc, mid_cb=None):
    """S^T/exp/PV + sums & ctx drains for one 512-token group (both heads).

    The two heads' S^T matmuls are row-group packed: head h's K=64
    contraction occupies array rows 64h..64h+63, so the pair runs
    concurrently on the PE (measured ~2.7x over sequential issue).

    With mid_cb set, all 16 ST/exp pairs are emitted first, then mid_cb()
    (used for the V projection: ScalarE stays busy on the exps while the
    PE waits for vT's DMA), then the PV accumulation.
    """
    nc = E.nc
    btok = b * T
    tq0 = btok + tqc * 512

    sums_h = [
        E.bc_pool.tile([1, 512], FP32, tag=f"sums{h}", name=f"sums{h}")
        for h in range(2)
    ]
    ctx2 = [
        E.ctx_ps.tile([65, 512], FP32, tag="ctx", name=f"ctx{h}")
        for h in range(2)
    ]

    def st_exp(tk):
        st = E.st_ps.tile([128, 2, 512], FP32, tag="st", name="st")
        for h in range(2):
            nc.tensor.matmul(
                st[:, h, :],
                E.kT_sb[h * 64:(h + 1) * 64,
                        btok + tk * 128: btok + (tk + 1) * 128],
                E.qT_sb[h * 64:(h + 1) * 64, tq0:tq0 + 512],
                start=True,
                stop=True,
            )
        pt = E.pt_pool.tile([128, 2, 512], BF16, tag="pt", name="pt")
        nc.scalar.activation(pt[:], st[:], ACT_EXP, scale=SCALE)
        return pt

    def pv(tk, pt):
        for h in range(2):
            nc.tensor.matmul(
                ctx2[h][:],
                E.vp[:, b * 16 + tk, h * 65:(h + 1) * 65],
                pt[:, h, :],
                start=(tk == 0),
                stop=(tk == 15),
            )

    if mid_cb is None:
        for tk in range(16):
            pv(tk, st_exp(tk))
    else:
        pts = [st_exp(tk) for tk in range(16)]
        mid_cb()
        for tk in range(16):
            pv(tk, pts[tk])

    for h in range(2):
        # softmax sums (PSUM row 64) -> sums tile partition 0
        nc.vector.tensor_copy(sums_h[h][0:1, :], ctx2[h][64:65, :])
        # ctx drain with bf16 cast (h1 shifts base 0 -> 64)
        nc.vector.tensor_copy(
            E.ctxT[h * 64:(h + 1) * 64, tq0:tq0 + 512], ctx2[h][0:64, :])
    return (tq0, sums_h)


def _norm_outproj(E, tq0, sums_h, out):
    """Normalization + V-bias + output projection for one 512-token group."""
    nc = E.nc
    bcast = E.bc_pool.tile([128, 512], FP32, tag="bcast")
    bcb = E.bc_pool.tile([128, 512], FP32, tag="bcb")
    nc.gpsimd.partition_broadcast(bcast[0:64, :], sums_h[0][0:1, :])
    nc.gpsimd.partition_broadcast(bcb[0:64, :], sums_h[1][0:1, :])
    nc.vector.tensor_copy(bcast[64:128, :], bcb[0:64, :])
    recipb = E.bc_pool.tile([128, 512], FP32, tag="recipb")
    nc.vector.reciprocal_approx_fast(recipb[:], bcast[:])
    nc.vector.tensor_mul(E.ctxT[:, tq0:tq0 + 512], E.ctxT[:, tq0:tq0 + 512], recipb[:])
    nc.vector.tensor_scalar_add(
        E.ctxT[:, tq0:tq0 + 512], E.ctxT[:, tq0:tq0 + 512], E.bv_sb[:])

    # output projection for these 512 tokens
    for tc4 in range(4):
        t0 = tq0 + tc4 * 128
        for half in range(2):
            ops = E.po_ps.tile([128, 512], FP32, tag="po", name="ops")
            nc.tensor.matmul(
                ops[:],
                E.ctxT[:, t0:t0 + 128],
                E.wo_sb[:, half * 512:(half + 1) * 512],
                start=True,
                stop=True,
            )
            ostg = E.ostg_pool.tile([128, 512], FP32, tag="ostg")
            nc.vector.tensor_copy(ostg[:], ops[:])
            nc.sync.dma_start(
                out.ap()[t0:t0 + 128, half * 512:(half + 1) * 512], ostg[:])


# ---------------- host-side helpers ----------------

def core_inputs(q, k, v, Wq, bq_, Wk, bk_, Wv, bv_, Wo, core):
    """Build the per-core input map (numpy, host-side shard/layout prep)."""
    import ml_dtypes

    bf16 = ml_dtypes.bfloat16
    dsl = slice(core * PH, (core + 1) * PH)
    return {
        "wq": np.ascontiguousarray(Wq[:, dsl]).astype(bf16),
        "wk": np.ascontiguousarray(Wk[:, dsl]).astype(bf16),
        "wv": np.ascontiguousarray(Wv[:, dsl]).astype(bf16),
        "wo": np.ascontiguousarray(Wo[dsl, :]).astype(bf16),
        "bq": np.ascontiguousarray(bq_[dsl]).reshape(PH, 1).astype(np.float32),
        "bk": np.ascontiguousarray(bk_[dsl]).reshape(PH, 1).astype(np.float32),
        "bv": np.ascontiguousarray(bv_[dsl]).reshape(PH, 1).astype(np.float32),
    }


def shared_inputs(q, k, v):
    import ml_dtypes

    bf16 = ml_dtypes.bfloat16
    qT_np = np.ascontiguousarray(q.reshape(N, D).T).astype(bf16)
    kT_np = np.ascontiguousarray(k.reshape(N, D).T).astype(bf16)
    vT_np = np.ascontiguousarray(v.reshape(N, D).T).astype(bf16)
    return {"qT": qT_np, "kT": kT_np, "vT": vT_np}


# ---------------- public entry point ----------------

_NC_CACHE = []


def _get_nc():
    if not _NC_CACHE:
        _NC_CACHE.append(build())
    return _NC_CACHE[0]


def kernel(q, k, v, Wq, bq, Wk, bk, Wv, bv, Wo, bo):
    from concourse import bass_utils

    q = np.asarray(q, np.float32)
    k = np.asarray(k, np.float32)
    v = np.asarray(v, np.float32)
    Wq, bq = np.asarray(Wq, np.float32), np.asarray(bq, np.float32)
    Wk, bk = np.asarray(Wk, np.float32), np.asarray(bk, np.float32)
    Wv, bv = np.asarray(Wv, np.float32), np.asarray(bv, np.float32)
    Wo, bo = np.asarray(Wo, np.float32), np.asarray(bo, np.float32)

    nc = _get_nc()
    shared = shared_inputs(q, k, v)
    in_maps = []
    for core in range(8):
        m = dict(shared)
        m.update(core_inputs(q, k, v, Wq, bq, Wk, bk, Wv, bv, Wo, core))
        in_maps.append(m)

    res = bass_utils.run_bass_kernel_spmd(nc, in_maps, core_ids=list(range(8)))

    acc = np.zeros((N, D), np.float64)
    for r in res.results:
        acc += r["out"].astype(np.float64)
    outp = (acc + bo.astype(np.float64)).astype(np.float32)
    return outp.reshape(B, T, D)

